# revision 14
# baseline (speedup 1.0000x reference)
"""Trainium2 Bass kernel for nn_CompressiveMemory_57750130262084.

The reference computes (B=8, S=4096, DK=DV=1024):
    sigma  = elu(query) + 1                                  [B,S,DK]
    memory = einsum('bkd,bsv->bkv', swap(sigma), value)      [B,DK,DV]
    z_norm = sum_s sigma                                     [B,DK]
    out    = einsum('bsd,bkv->bsv', sigma, memory)
           / einsum('bsd,bk->bs',  sigma, z_norm)[..., None]

Every einsum uses disjoint summed subscripts, so each factorises into
outer products of independent reductions:
    memory[b,k,v]    = z_norm[b,k] * VS[b,v]      with VS[b,v] = sum_s value[b,s,v]
    retrieved[b,s,v] = rs[b,s] * Z[b] * VS[b,v]   with rs = rowsum(sigma), Z = sum_k z_norm
    denom[b,s]       = rs[b,s] * Z[b]
    out[b,s,v]       = VS[b,v]                    (exactly; query cancels)

So the kernel is a column-sum of `value` over S, broadcast over S.
Sharding: data-parallel over batch, one NeuronCore per batch element.

Schedule per core (v6).  TRN2 has 16 DMA queues at ~26 GB/s each
(~417 GB/s aggregate, descriptor-size independent), so the floor is
read 16.8 MB + write fp16 8.4 MB back-to-back = ~60 us of DMA plus
the ~8.5 us NEFF prologue and the reduction tail.  This version
minimises the tail:
  - p-major input layout: partition p holds 32 CONTIGUOUS DRAM rows
    [32p, 32p+32); row placement is irrelevant (everything is summed).
  - input split across BOTH HWDGE engines (SP + Activation), rows
    0..15 / 16..31, transfers per engine of [1,1,2,2,2,2,2,2,1,.5,.5]
    rows: 2-row middles keep the DVE completely queue-free, the last
    two arrivals are 0.5 MB half-row pairs.
  - pair k = (sync row k, scalar row k+16) lands atomically; the DVE
    adds each pair into a tmp ring CASTING TO BF16, so the PE's
    PSUM-accumulating ones^T @ tmp (partition-reduce + broadcast)
    costs 1 HW pass per 512-bank instead of f32's 2.  Both engines
    run at data-arrival with zero backlog; the final half-row pieces
    pipeline add -> 1-pass matmul per bank with per-bank stop.
    bf16 pair rounding contributes ~8e-4 relative error (tolerance
    is 2e-2; fp32 keeps the partition/psum accumulation exact).
  - output stored as float16 (host upcasts), halving write traffic.
    PSUM is drained ONCE per bank by ACT (PSUM readers serialize
    anyway) and replicated from SBUF by DVE, giving two adjacent
    copies so output descriptors cover REP=2 rows (4 KB).  ACT's
    activation-table load is pre-warmed at t~0.
  - output DMAs alternate between the two HWDGE engines.
"""

import numpy as np

B, S, D = 8, 4096, 1024
P = 128                 # SBUF partitions
RPP = S // P            # 32 rows per partition (p-major layout)
# rows/partition per transfer, in units of half-rows (per engine)
GROUPS_HALF = [8, 8, 8, 4, 2, 1, 1]
TMP_SLOTS = 4
REP = 2                 # output row-replicas in SBUF -> 4KB descriptors
N_OUT = 4               # output transfers (1024 rows each)
H = 512                 # PSUM bank width in f32 (matmul N limit)

_CACHE: dict = {}


def _build_program():
    import concourse.mybir as mybir
    import concourse.tile as tile
    from concourse import bacc

    f32 = mybir.dt.float32
    f16 = mybir.dt.float16
    bf16 = mybir.dt.bfloat16
    assert sum(GROUPS_HALF) == 32
    nc = bacc.Bacc("TRN2", target_bir_lowering=False, debug=False, num_devices=B, enable_asserts=False)
    v = nc.declare_dram_parameter("value", [S, D], f32, isOutput=False)
    o = nc.declare_dram_parameter("out", [S, D], f16, isOutput=True)

    v_pm = v[:].rearrange("(p r) m -> p (r m)", p=P)       # [128][32*1024]
    # [4][128][4][2048]: transfer i covers 1024 rows; descriptor (p, j)
    # is REP=2 consecutive DRAM rows (4 KB) reading the same SBUF 4 KB.
    o_re = o[:].rearrange("(i j p n) m -> i p j (n m)", p=P, n=REP, j=4)

    with tile.TileContext(nc) as tc:
        with (
            tc.tile_pool(name="in", bufs=1) as in_pool,
            tc.tile_pool(name="tmp", bufs=1) as tmp_pool,
            tc.tile_pool(name="ones", bufs=1) as ones_pool,
            tc.tile_pool(name="bcast", bufs=1) as bcast_pool,
            tc.tile_pool(name="warm", bufs=1) as warm_pool,
            tc.tile_pool(name="psum", bufs=1, space="PSUM") as psum_pool,
        ):
            ones = ones_pool.tile([P, P], bf16)
            nc.vector.memset(ones[:], 1.0)
            warm = warm_pool.tile([P, 16], f32)
            nc.scalar.copy(warm[:], ones[:, 0:16])   # pre-warm ACT table load

            t = in_pool.tile([P, RPP * D], f32)
            tmp = tmp_pool.tile([P, TMP_SLOTS * D], bf16)
            ps = psum_pool.tile([P, D], f32)

            # Input DMAs: each engine issues its transfers back-to-back.
            for half, eng in ((0, nc.sync), (1, nc.scalar)):
                h0 = half * 32                       # offset in half-rows
                for g in GROUPS_HALF:
                    sl = slice(h0 * H, (h0 + g) * H)
                    eng.dma_start(t[:, sl], v_pm[:, sl])
                    h0 += g

            # Pairs 0..14: DVE add (f32 -> bf16 tmp), PE 1-pass-per-bank
            # PSUM accumulation.
            for k in range(15):
                a = t[:, k * D : (k + 1) * D]
                b = t[:, (k + 16) * D : (k + 17) * D]
                tk = tmp[:, (k % TMP_SLOTS) * D : (k % TMP_SLOTS + 1) * D]
                nc.vector.tensor_add(tk, a, b)
                for h in range(2):
                    nc.tensor.matmul(
                        ps[:, h * H : (h + 1) * H],
                        ones[:],
                        tk[:, h * H : (h + 1) * H],
                        start=(k == 0),
                        stop=False,
                    )

            # Pair 15 arrives as two half-row pieces; per-bank pipeline
            # with per-bank stop so each drain starts as soon as its
            # bank settles.
            for h in range(2):
                a = t[:, 15 * D + h * H : 15 * D + (h + 1) * H]
                b = t[:, 31 * D + h * H : 31 * D + (h + 1) * H]
                th = tmp[:, 3 * D + h * H : 3 * D + (h + 1) * H]
                nc.vector.tensor_add(th, a, b)
                nc.tensor.matmul(
                    ps[:, h * H : (h + 1) * H],
                    ones[:],
                    th,
                    start=False,
                    stop=True,
                )

            # Drain PSUM once per bank (ACT), replicate from SBUF (DVE).
            bc = bcast_pool.tile([P, REP * D], f16)
            nc.scalar.copy(bc[:, 0:H], ps[:, 0:H])
            nc.vector.tensor_copy(bc[:, D : D + H], bc[:, 0:H])
            nc.scalar.copy(bc[:, H:D], ps[:, H:D])
            nc.vector.tensor_copy(bc[:, D + H : 2 * D], bc[:, H:D])

            src = bc[:].unsqueeze(1).to_broadcast((P, 4, REP * D))
            for i in range(N_OUT):
                eng = nc.sync if i % 2 == 0 else nc.scalar
                eng.dma_start(o_re[i], src)

    nc.compile()
    return nc


def _get_program():
    if "nc" not in _CACHE:
        _CACHE["nc"] = _build_program()
    return _CACHE["nc"]


def kernel(query: np.ndarray, value: np.ndarray) -> np.ndarray:
    from concourse.bass_utils import run_bass_kernel_spmd

    del query  # output is exactly independent of query (see module docstring)
    value = np.ascontiguousarray(value, dtype=np.float32)
    assert value.shape == (B, S, D)

    nc = _get_program()
    in_maps = [{"value": value[b]} for b in range(B)]
    try:
        res = run_bass_kernel_spmd(nc, in_maps, list(range(B)))
    except Exception:
        # The tunneled runtime occasionally surfaces a transient
        # NRT_EXEC_UNIT_UNRECOVERABLE on the first dispatch; retry once.
        import time

        time.sleep(2.0)
        res = run_bass_kernel_spmd(nc, in_maps, list(range(B)))
    return np.stack(
        [res.results[b]["out"].astype(np.float32) for b in range(B)], axis=0
    )


# revision 15
# speedup vs baseline: 1.0260x; 1.0260x over previous
"""Trainium2 Bass kernel for nn_CompressiveMemory_57750130262084.

The reference computes (B=8, S=4096, DK=DV=1024):
    sigma  = elu(query) + 1                                  [B,S,DK]
    memory = einsum('bkd,bsv->bkv', swap(sigma), value)      [B,DK,DV]
    z_norm = sum_s sigma                                     [B,DK]
    out    = einsum('bsd,bkv->bsv', sigma, memory)
           / einsum('bsd,bk->bs',  sigma, z_norm)[..., None]

Every einsum uses disjoint summed subscripts, so each factorises into
outer products of independent reductions:
    memory[b,k,v]    = z_norm[b,k] * VS[b,v]      with VS[b,v] = sum_s value[b,s,v]
    retrieved[b,s,v] = rs[b,s] * Z[b] * VS[b,v]   with rs = rowsum(sigma), Z = sum_k z_norm
    denom[b,s]       = rs[b,s] * Z[b]
    out[b,s,v]       = VS[b,v]                    (exactly; query cancels)

So the kernel is a column-sum of `value` over S, broadcast over S.
Sharding: data-parallel over batch, one NeuronCore per batch element.

Schedule per core (v6).  TRN2 has 16 DMA queues at ~26 GB/s each
(~417 GB/s aggregate, descriptor-size independent), so the floor is
read 16.8 MB + write fp16 8.4 MB back-to-back = ~60 us of DMA plus
the ~8.5 us NEFF prologue and the reduction tail.  This version
minimises the tail:
  - p-major input layout: partition p holds 32 CONTIGUOUS DRAM rows
    [32p, 32p+32); row placement is irrelevant (everything is summed).
  - input split across BOTH HWDGE engines (SP + Activation), rows
    0..15 / 16..31, transfers per engine of [1,1,2,2,2,2,2,2,1,.5,.5]
    rows: 2-row middles keep the DVE completely queue-free, the last
    two arrivals are 0.5 MB half-row pairs.
  - pair k = (sync row k, scalar row k+16) lands atomically; the DVE
    adds each pair into a tmp ring CASTING TO BF16, so the PE's
    PSUM-accumulating ones^T @ tmp (partition-reduce + broadcast)
    costs 1 HW pass per 512-bank instead of f32's 2.  Both engines
    run at data-arrival with zero backlog; the final half-row pieces
    pipeline add -> 1-pass matmul per bank with per-bank stop.
    bf16 pair rounding contributes ~8e-4 relative error (tolerance
    is 2e-2; fp32 keeps the partition/psum accumulation exact).
  - output stored as float16 (host upcasts), halving write traffic.
    PSUM is drained ONCE per bank by ACT (PSUM readers serialize
    anyway) and replicated from SBUF by DVE, giving two adjacent
    copies so output descriptors cover REP=2 rows (4 KB).  ACT's
    activation-table load is pre-warmed at t~0.
  - output DMAs alternate between the two HWDGE engines.
"""

import numpy as np

B, S, D = 8, 4096, 1024
P = 128                 # SBUF partitions
RPP = S // P            # 32 rows per partition (p-major layout)
# rows/partition per transfer, in units of half-rows (per engine)
GROUPS_HALF = [8, 8, 8, 4, 2, 1, 1]
TMP_SLOTS = 4
REP = 2                 # output row-replicas in SBUF -> 4KB descriptors
N_OUT = 4               # output transfers (1024 rows each)
H = 512                 # PSUM bank width in f32 (matmul N limit)

_CACHE: dict = {}


def _build_program():
    import concourse.mybir as mybir
    import concourse.tile as tile
    from concourse import bacc

    f32 = mybir.dt.float32
    f16 = mybir.dt.float16
    bf16 = mybir.dt.bfloat16
    assert sum(GROUPS_HALF) == 32
    nc = bacc.Bacc("TRN2", target_bir_lowering=False, debug=False, num_devices=B, enable_asserts=False)
    v = nc.declare_dram_parameter("value", [S, D], f32, isOutput=False)
    o = nc.declare_dram_parameter("out", [S, D], f16, isOutput=True)

    v_pm = v[:].rearrange("(p r) m -> p (r m)", p=P)       # [128][32*1024]
    # [4][128][4][2048]: transfer i covers 1024 rows; descriptor (p, j)
    # is REP=2 consecutive DRAM rows (4 KB) reading the same SBUF 4 KB.
    o_re = o[:].rearrange("(i j p n) m -> i p j (n m)", p=P, n=REP, j=4)

    with tile.TileContext(nc) as tc:
        with (
            tc.tile_pool(name="in", bufs=1) as in_pool,
            tc.tile_pool(name="tmp", bufs=1) as tmp_pool,
            tc.tile_pool(name="ones", bufs=1) as ones_pool,
            tc.tile_pool(name="bcast", bufs=1) as bcast_pool,
            tc.tile_pool(name="warm", bufs=1) as warm_pool,
            tc.tile_pool(name="psum", bufs=1, space="PSUM") as psum_pool,
        ):
            ones = ones_pool.tile([P, P], bf16)
            nc.vector.memset(ones[:], 1.0)
            warm = warm_pool.tile([P, 16], f32)
            nc.scalar.copy(warm[:], ones[:, 0:16])   # pre-warm ACT table load

            t = in_pool.tile([P, RPP * D], f32)
            tmp = tmp_pool.tile([P, TMP_SLOTS * D], bf16)
            ps = psum_pool.tile([P, D], f32)

            # Input DMAs: each engine issues its transfers back-to-back.
            for half, eng in ((0, nc.sync), (1, nc.scalar)):
                h0 = half * 32                       # offset in half-rows
                for g in GROUPS_HALF:
                    sl = slice(h0 * H, (h0 + g) * H)
                    eng.dma_start(t[:, sl], v_pm[:, sl])
                    h0 += g

            # Pairs 0..14: DVE add (f32 -> bf16 tmp), PE 1-pass-per-bank
            # PSUM accumulation.
            for k in range(15):
                a = t[:, k * D : (k + 1) * D]
                b = t[:, (k + 16) * D : (k + 17) * D]
                tk = tmp[:, (k % TMP_SLOTS) * D : (k % TMP_SLOTS + 1) * D]
                nc.vector.tensor_add(tk, a, b)
                for h in range(2):
                    nc.tensor.matmul(
                        ps[:, h * H : (h + 1) * H],
                        ones[:],
                        tk[:, h * H : (h + 1) * H],
                        start=(k == 0),
                        stop=False,
                    )

            # Pair 15 arrives as two half-row pieces; per-bank pipeline
            # with per-bank stop so each drain starts as soon as its
            # bank settles.
            for h in range(2):
                a = t[:, 15 * D + h * H : 15 * D + (h + 1) * H]
                b = t[:, 31 * D + h * H : 31 * D + (h + 1) * H]
                th = tmp[:, 3 * D + h * H : 3 * D + (h + 1) * H]
                nc.vector.tensor_add(th, a, b)
                nc.tensor.matmul(
                    ps[:, h * H : (h + 1) * H],
                    ones[:],
                    th,
                    start=False,
                    stop=True,
                )

            # Drain PSUM once per bank on DIFFERENT engines (each waits
            # only its own bank's stop), then cross-replicate from SBUF.
            bc = bcast_pool.tile([P, REP * D], f16)
            nc.scalar.copy(bc[:, 0:H], ps[:, 0:H])
            nc.vector.tensor_copy(bc[:, H:D], ps[:, H:D])
            nc.vector.tensor_copy(bc[:, D : D + H], bc[:, 0:H])
            nc.scalar.copy(bc[:, D + H : 2 * D], bc[:, H:D])

            src = bc[:].unsqueeze(1).to_broadcast((P, 4, REP * D))
            for i in range(N_OUT):
                eng = nc.sync if i % 2 == 0 else nc.scalar
                eng.dma_start(o_re[i], src)

    nc.compile()
    return nc


def _get_program():
    if "nc" not in _CACHE:
        _CACHE["nc"] = _build_program()
    return _CACHE["nc"]


def kernel(query: np.ndarray, value: np.ndarray) -> np.ndarray:
    from concourse.bass_utils import run_bass_kernel_spmd

    del query  # output is exactly independent of query (see module docstring)
    value = np.ascontiguousarray(value, dtype=np.float32)
    assert value.shape == (B, S, D)

    nc = _get_program()
    in_maps = [{"value": value[b]} for b in range(B)]
    try:
        res = run_bass_kernel_spmd(nc, in_maps, list(range(B)))
    except Exception:
        # The tunneled runtime occasionally surfaces a transient
        # NRT_EXEC_UNIT_UNRECOVERABLE on the first dispatch; retry once.
        import time

        time.sleep(2.0)
        res = run_bass_kernel_spmd(nc, in_maps, list(range(B)))
    return np.stack(
        [res.results[b]["out"].astype(np.float32) for b in range(B)], axis=0
    )
